# revision 22
# baseline (speedup 1.0000x reference)
"""Trainium2 8-core Bass kernel for the SKalmanNet dense-MLP GEMV chain.

Network (batch=1):
  x   = concat(state_inno, precov, residual, meas_cov)          [128]
  l1  = relu(W1 @ x + b1)                                       [1344]
  gi  = w_ih @ l1 + b_ih ; gh = w_hh @ h0 + b_hh                [12288]
  r,z = sigmoid(gi+gh) gates ; n = tanh(gi_n + r*gh_n)
  h   = (1-z)*n + z*h0                                          [4096]
  x_hat = W2b @ relu(W2a @ h + b2a) + b2b                       [32]
  P_hat = W3b @ relu(W3a @ h + b3a) + b3b                       [32]

Sharding: every large matrix is row-sharded (output dim) across 8 cores;
W1 is replicated (tiny) so l1 needs no collective. The only collective is
one 16KB AllGather of h. The final 32-vector partials (W2b/W3b column
shards) are summed on the host during unsharding.

Layouts: activations live as "stationary" columns [128, nblk] so they can
be the matmul lhsT; weights are host-pre-transposed so W.T tiles stream
as the rhs. All biases are folded into the matmuls via an augmented
contraction element that is constant 1.
"""

import os
import sys

sys.path.insert(0, "/opt/trn_rl_repo")

import numpy as np
import ml_dtypes

# ---------------------------------------------------------------- constants
NCORES = 8
X_DIM = 32
IN2 = 128                      # l1 input dim
H1 = 1344                      # l1 output / GRU input dim
H1P = 1408                     # padded to 11*128 (pad block holds the bias row)
GH = 4096                      # GRU hidden
GHP = 4224                     # padded to 33*128 (aug block holds bias row)
H2 = 4096                      # head hidden
SH = 512                       # per-core hidden slice (GH/8 == H2/8)
K1 = H1P // 128                # 11 contraction blocks for gi
KH = GHP // 128                # 33 contraction blocks for gh / heads
KF = 640 // 128                # 5 contraction blocks for the final gemv

GRU_CHUNK = 4                  # k-blocks per DMA chunk for wiht/whht
HEAD_CHUNK = 8                 # k-blocks per DMA chunk for w2at/w3at

_WDT_NAME = os.environ.get("KERNEL_DTYPE", "bf16")
_GDT_NAME = os.environ.get("KERNEL_GRU_DT", "f8e3")     # bf16 | f8e3
_HDT_NAME = os.environ.get("KERNEL_HEAD_DT", "f8e3")    # bf16 | f8e3
_GATHER = os.environ.get("KERNEL_GATHER", "bcast2")
_EARLY_PREP = os.environ.get("KERNEL_EARLY_PREP", "0") == "1"
_TAIL_READBACK = os.environ.get("KERNEL_TAIL_READBACK", "1") == "1"
_RENDEZVOUS = os.environ.get("KERNEL_RENDEZVOUS", "1") == "1"
FP8_TARGET = 12.0               # quantization scale target (e3m4 max 15.5)

_compiled = {}


def _np_wdt():
    return {"bf16": ml_dtypes.bfloat16, "f32": np.float32, "f32r": np.float32}[
        _WDT_NAME
    ]


def _build(wdt_name, gdt_name, hdt_name, gather):
    import concourse.bass as bass  # noqa: F401
    import concourse.mybir as mybir
    import concourse.tile as tile
    from concourse import bacc

    F32 = mybir.dt.float32
    WDT = {
        "bf16": mybir.dt.bfloat16,
        "f32": mybir.dt.float32,
        "f32r": mybir.dt.float32r,
    }[wdt_name]
    GDT = {"bf16": mybir.dt.bfloat16, "f8e3": mybir.dt.float8e3}[gdt_name]
    HDT = {"bf16": mybir.dt.bfloat16, "f8e3": mybir.dt.float8e3}[hdt_name]
    AF = mybir.ActivationFunctionType
    ALU = mybir.AluOpType
    ts = bass.ts

    nc = bacc.Bacc("TRN2", target_bir_lowering=False, debug=False, num_devices=NCORES)

    # ------------------------------------------------------------- I/O decl
    # All small constants ride in two packed blobs (per-partition contiguous)
    # so they cost ~256 DMA descriptors instead of ~2000 tiny ones that
    # starve behind the weight stream on the shared DMA engines.
    CW = H1P + KH + 2 * KF * 32 + 1      # w1t | h0stat | w2b | w3b | x
    CF = K1 + 2                          # b1s | one | hsc
    OFF_H0S = H1P
    OFF_W2B = H1P + KH
    OFF_W3B = H1P + KH + KF * 32
    OFF_X = CW - 1
    blobw = nc.dram_tensor("blobw", [128, CW], WDT, kind="ExternalInput")
    blobf = nc.dram_tensor("blobf", [128, CF], F32, kind="ExternalInput")
    # packed streams: per chunk, [128, nkb*N] with per-partition-contiguous
    # bytes so each DMA descriptor is one long run
    wihp = nc.dram_tensor("wihp", [K1 * 128 * 3 * SH], GDT, kind="ExternalInput")
    whhp = nc.dram_tensor("whhp", [KH * 128 * 3 * SH], GDT, kind="ExternalInput")
    w2ap = nc.dram_tensor("w2ap", [KH * 128 * SH], HDT, kind="ExternalInput")
    w3ap = nc.dram_tensor("w3ap", [KH * 128 * SH], HDT, kind="ExternalInput")
    h0row = nc.dram_tensor("h0row", [1, SH], F32, kind="ExternalInput")
    coreid = nc.dram_tensor("coreid", [1, 1], mybir.dt.uint32, kind="ExternalInput")
    out = nc.dram_tensor("out", [1, 64], F32, kind="ExternalOutput")

    def chunks_of(t, nkb_total, step, width):
        out = []
        for kb0 in range(0, nkb_total, step):
            out.append((t, kb0, min(step, nkb_total - kb0), width))
        return out

    gru_chunks = chunks_of(wihp, K1, GRU_CHUNK, 3 * SH) + chunks_of(
        whhp, KH, GRU_CHUNK, 3 * SH
    )
    n_wih_chunks = len(chunks_of(wihp, K1, GRU_CHUNK, 3 * SH))
    head_chunks = chunks_of(w2ap, KH, HEAD_CHUNK, SH) + chunks_of(
        w3ap, KH, HEAD_CHUNK, SH
    )

    with tile.TileContext(nc) as tc:
        with (
            tc.tile_pool(name="const", bufs=1) as cp,
            tc.tile_pool(name="gru", bufs=12) as gp,
            tc.tile_pool(name="head", bufs=10) as hp,
            tc.tile_pool(name="acts", bufs=1) as ap,
            tc.tile_pool(name="dram", bufs=1, space="DRAM") as dp,
        ):
            # ------------------------------------------------ constant loads
            # Constants go on the ACT HWDGE ring so the SP ring starts the
            # big weight stream immediately.
            bw_sb = cp.tile([128, CW], WDT, tag="bw")
            nc.scalar.dma_start(bw_sb[:], blobw[:])
            bf_sb = cp.tile([128, CF], F32, tag="bf")
            nc.scalar.dma_start(bf_sb[:], blobf[:])
            h0r_sb = cp.tile([1, SH], F32, tag="h0r")
            nc.scalar.dma_start(h0r_sb[:], h0row[:])
            cid_sb = cp.tile([1, 1], mybir.dt.uint32, tag="cid")
            nc.scalar.dma_start(cid_sb[:], coreid[:])
            def ncfw_rendezvous(read_back):
                # All-core ncfw AllReduce. Its presence in the NEFF makes the
                # runtime align the 8 cores' launches (without it they stagger
                # by ~100us/core). ncfw's CC core has a ~65us wake latency
                # after the trigger: trigger at program start, and (bcast2)
                # read nothing back so no queue ever blocks on completion —
                # the mesh completes in the background well before the NEFF
                # drains.
                bar_sb = cp.tile([1, 8], mybir.dt.uint32, tag="bar")
                nc.vector.memset(bar_sb[:], 1)
                bar_in = dp.tile([1, 8], mybir.dt.uint32, name="bar_in")
                bar_out = dp.tile([1, 8], mybir.dt.uint32, name="bar_out")
                if _RENDEZVOUS:
                    # bar_in rides the ACT HWDGE ring: a gpsimd (SWDGE) dma
                    # here posts its completion sem only at ~60-74us, and the
                    # collective enqueue + the gather critical queue up behind
                    # it on the Pool queue.
                    nc.scalar.dma_start(bar_in[:], bar_sb[:])
                    nc.gpsimd.collective_compute(
                        "AllReduce",
                        mybir.AluOpType.add,
                        replica_groups=[list(range(NCORES))],
                        ins=[bar_in[:].opt()],
                        outs=[bar_out[:].opt()],
                    )
                bar2_sb = cp.tile([1, 8], mybir.dt.uint32, tag="bar2")
                if read_back:
                    # cc/bcast: gather logic reads bar2 -> load now (gpsimd).
                    nc.gpsimd.dma_start(bar2_sb[:], bar_out[:])
                return bar_out, bar2_sb

            # gather target: written remotely by all 8 cores' broadcasts.
            # memset early so the slot is reserved for the whole kernel and
            # cannot alias a streaming tile when a peer's write lands.
            h_sb = ap.tile([128, KH], WDT, tag="hstat")
            hloc = ap.tile([128, 4], WDT, tag="hloc")
            if gather == "bcast2":
                # ncfw-free gather. Startup: zero h_sb, then check in with all
                # peers via a remote-sem broadcast (the flag barrier), and
                # pre-generate the h-broadcast descriptors (If-ladder) OFF the
                # critical path. bass defers the prep's data deps to the
                # trigger, so at gather time only trigger+transfer remain.
                # The alignment collective is emitted first so its trigger
                # lands at ~9us; nothing reads its output until the gpsimd
                # queue tail.
                bar_out, bar2_sb = ncfw_rendezvous(read_back=False)
                bar_r = nc.alloc_semaphore("bar_r")
                bar_l = nc.alloc_semaphore("bar_l")
                bar_p = nc.alloc_semaphore("bar_prep_sem")
                psem = nc.alloc_semaphore("bc_prep_sem")
                lsem = nc.alloc_semaphore("bc_local_sem")
                rsem = nc.alloc_semaphore("bc_remote_sem")
                nc.vector.memset(hloc[:], 0.0)
                nc.vector.memset(h_sb[:], 0.0)
                # NOTE: no startup tile_critical — a critical here makes every
                # later instruction (via pool boundaries) wait on the Pool
                # queue's exit, and the gpsimd SWDGE ucode library load inside
                # costs ~18us, stalling the PE start at ~32us. The flag
                # barrier moved into the gather critical below.
                if _EARLY_PREP:
                    with tc.tile_critical(sync_engine=mybir.EngineType.Pool, no_gpsimd_drain=True):
                        eng = nc.gpsimd
                        reg = eng.alloc_register("cid_reg")
                        eng.reg_load(reg, cid_sb[0:1, 0:1])
                        for c in range(NCORES):
                            with eng.If_eq(reg, c):
                                eng.remote_dma_broadcast(
                                    out_ap=h_sb[:, c * 4 : (c + 1) * 4],
                                    in_ap=hloc[:],
                                    remote_sem=rsem,
                                    local_sem=lsem,
                                    rdests=[(0, k) for k in range(NCORES)],
                                ).then_inc(psem, 1)
                            with eng.Else():
                                eng.nop()
            else:
                nc.vector.memset(h_sb[:], 0.0)


            if gather != "bcast2":
                bar_out, bar2_sb = ncfw_rendezvous(read_back=True)

            # ------------------------------------- weight stream DMAs (HWDGE)
            # Consumption order: wih -> whh -> heads. The GRU (and hence the
            # gather) completes as early as possible; the gather + gate tail
            # hides under the head-weight stream.
            def stream_chunk(pool, spec, tag, engine, dt):
                t, kb0, nkb, width = spec
                g = pool.tile([128, GRU_CHUNK * 3 * SH] if width == 3 * SH
                              else [128, HEAD_CHUNK * SH], dt, tag=tag, name=tag)
                off = kb0 * 128 * width
                sz = nkb * 128 * width
                src_ap = t[off : off + sz].rearrange("(p x) -> p x", p=128)
                engine.dma_start(g[:, 0 : nkb * width], src_ap)
                return g

            gru_tiles = []
            for spec in gru_chunks:
                gru_tiles.append(stream_chunk(gp, spec, "gruw", nc.sync, GDT))
            head_tiles = []
            for spec in head_chunks:
                head_tiles.append(stream_chunk(hp, spec, "headw", nc.sync, HDT))

            with tc.tile_pool(name="psA", bufs=1, space="PSUM") as psA:
                # ------------------------------------------- L1 (W-stationary)
                l1p = psA.tile([128, K1], F32, tag="l1p")
                for j in range(K1):
                    nc.tensor.matmul(
                        l1p[:, j : j + 1],
                        bw_sb[:, ts(j, 128)],
                        bw_sb[:, OFF_X : OFF_X + 1],
                        start=True,
                        stop=True,
                    )
                l1t = ap.tile([128, K1], F32, tag="l1t")
                nc.vector.scalar_tensor_tensor(
                    l1t[:], l1p[:], 1.0, bf_sb[:, 0:K1], ALU.mult, ALU.add
                )
                l1_sb = ap.tile([128, K1], WDT, tag="l1s")
                nc.scalar.activation(l1_sb[:], l1t[:], AF.Relu)

                # ------------------------------------------- GRU matmuls
                gi = [psA.tile([1, SH], F32, tag=f"gi{g}", name=f"gi{g}") for g in range(3)]
                gh = [psA.tile([1, SH], F32, tag=f"gh{g}", name=f"gh{g}") for g in range(3)]
                for ci, (t, kb0, nkb, width) in enumerate(gru_chunks):
                    is_ih = ci < n_wih_chunks
                    dst = gi if is_ih else gh
                    klast = (K1 if is_ih else KH) - 1
                    for kk in range(nkb):
                        kb = kb0 + kk
                        stat = (
                            l1_sb[:, kb : kb + 1]
                            if is_ih
                            else bw_sb[:, OFF_H0S + kb : OFF_H0S + kb + 1]
                        )
                        for g in range(3):
                            base = kk * width + g * SH
                            nc.tensor.matmul(
                                dst[g][:],
                                stat,
                                gru_tiles[ci][:, base : base + SH],
                                start=(kb == 0),
                                stop=(kb == klast),
                            )

                # gi -> SBUF (ScalarE; overlaps the gh matmul stream). DVE has
                # a single PSUM read port, so gate ops may touch <=1 PSUM operand.
                gis = ap.tile([1, 3 * SH], F32, tag="gis")
                for g in range(3):
                    nc.scalar.activation(gis[:, ts(g, SH)], gi[g][:], AF.Copy)

                # ------------------------------------------- gates (row layout)
                t_r = ap.tile([1, SH], F32, tag="gtmp", bufs=6)
                nc.vector.tensor_tensor(t_r[:], gis[:, ts(0, SH)], gh[0][:], ALU.add)
                r = ap.tile([1, SH], F32, tag="r")
                nc.scalar.activation(r[:], t_r[:], AF.Sigmoid)
                t_z = ap.tile([1, SH], F32, tag="gtmp", bufs=6)
                nc.vector.tensor_tensor(t_z[:], gis[:, ts(1, SH)], gh[1][:], ALU.add)
                z = ap.tile([1, SH], F32, tag="z")
                nc.scalar.activation(z[:], t_z[:], AF.Sigmoid)
                t_m = ap.tile([1, SH], F32, tag="gtmp", bufs=6)
                nc.vector.tensor_tensor(t_m[:], r[:], gh[2][:], ALU.mult)
                t_n = ap.tile([1, SH], F32, tag="gtmp", bufs=6)
                nc.vector.tensor_tensor(t_n[:], t_m[:], gis[:, ts(2, SH)], ALU.add)
                n_t = ap.tile([1, SH], F32, tag="n")
                nc.scalar.activation(n_t[:], t_n[:], AF.Tanh)
                t_d = ap.tile([1, SH], F32, tag="gtmp", bufs=6)
                nc.vector.tensor_tensor(t_d[:], h0r_sb[:], n_t[:], ALU.subtract)
                t_e = ap.tile([1, SH], F32, tag="gtmp", bufs=6)
                nc.vector.tensor_tensor(t_e[:], z[:], t_d[:], ALU.mult)
                h_row = ap.tile([1, SH], F32, tag="hrow")
                nc.vector.tensor_tensor(h_row[:], n_t[:], t_e[:], ALU.add)

            # ------------- h row -> stationary cols via rank-1 PE matmuls
            one = bf_sb[0:1, K1 : K1 + 1]  # constant 1.0
            hsc = bf_sb[0:1, K1 + 1 : K1 + 2]  # 1/s_head (1.0 unless heads fp8)
            with tc.tile_pool(name="psB", bufs=1, space="PSUM") as psB:
                hT4 = psB.tile([128, 4], F32, tag="hT4")
                for k in range(4):
                    nc.tensor.matmul(
                        hT4[:, k : k + 1],
                        h_row[0:1, ts(k, 128)],
                        hsc,
                        start=True,
                        stop=True,
                    )
                nc.vector.tensor_copy(hloc[:], hT4[:])

                # ---------------- all-gather h across the 8 cores
                h_use = ap.tile([128, KH], WDT, tag="huse")
                if gather == "bcast2" and _EARLY_PREP:
                    with tc.tile_critical(sync_engine=mybir.EngineType.Pool, no_gpsimd_drain=True):
                        eng = nc.gpsimd
                        eng.wait_ge(psem, 1)
                        eng.wait_ge(bar_r, 16)
                        eng.trigger_dma(count=1)
                        eng.wait_ge(lsem, 16)
                        eng.wait_ge(rsem, 16)
                        eng.tensor_copy(h_sb[0:1, 32:33], hsc)  # aug = 1/s_head
                    nc.vector.tensor_copy(h_use[:], h_sb[:])
                elif gather == "bcast2":
                    with tc.tile_critical(sync_engine=mybir.EngineType.Pool, no_gpsimd_drain=True):
                        eng = nc.gpsimd
                        # flag barrier: check in with all peers, then wait for
                        # everyone before firing the h broadcast (h_sb on all
                        # peers is long since zeroed by the early memset).
                        eng.remote_sem_update_broadcast(
                            bar_r, bar_l, rdests=[(0, k) for k in range(NCORES)]
                        ).then_inc(bar_p, 1)
                        eng.wait_ge(bar_p, 1)
                        eng.trigger_dma(count=1)
                        reg = eng.alloc_register("cid_reg")
                        eng.reg_load(reg, cid_sb[0:1, 0:1])
                        for c in range(NCORES):
                            with eng.If_eq(reg, c):
                                eng.remote_dma_broadcast(
                                    out_ap=h_sb[:, c * 4 : (c + 1) * 4],
                                    in_ap=hloc[:],
                                    remote_sem=rsem,
                                    local_sem=lsem,
                                    rdests=[(0, k) for k in range(NCORES)],
                                ).then_inc(psem, 1)
                            with eng.Else():
                                eng.nop()
                        eng.wait_ge(psem, 1)
                        eng.wait_ge(bar_r, 16)
                        eng.trigger_dma(count=1)
                        eng.wait_ge(lsem, 16)
                        eng.wait_ge(rsem, 16)
                        eng.tensor_copy(h_sb[0:1, 32:33], hsc)  # aug = 1/s_head
                    nc.vector.tensor_copy(h_use[:], h_sb[:])
                elif gather == "bcast":
                    psem = nc.alloc_semaphore("bc_prep_sem")
                    lsem = nc.alloc_semaphore("bc_local_sem")
                    rsem = nc.alloc_semaphore("bc_remote_sem")
                    with tc.tile_critical():
                        eng = nc.gpsimd
                        reg = eng.alloc_register("cid_reg")
                        # order after the startup alignment barrier
                        eng.reg_load(reg, bar2_sb[0:1, 0:1])
                        eng.reg_load(reg, cid_sb[0:1, 0:1])
                        for c in range(NCORES):
                            with eng.If_eq(reg, c):
                                eng.remote_dma_broadcast(
                                    out_ap=h_sb[:, c * 4 : (c + 1) * 4],
                                    in_ap=hloc[:],
                                    remote_sem=rsem,
                                    local_sem=lsem,
                                    rdests=[(0, k) for k in range(NCORES)],
                                ).then_inc(psem, 1)
                            with eng.Else():
                                eng.nop()
                        eng.wait_ge(psem, 1)
                        eng.trigger_dma(count=1)
                        eng.wait_ge(lsem, 16)
                        eng.wait_ge(rsem, 16)
                        eng.memset(h_sb[0:1, 32:33], 1.0)  # aug element
                        # copy into h_use so downstream consumers depend on
                        # the gathered data (remote writes invisible to Tile)
                        eng.tensor_copy(h_use[:], h_sb[:])
                else:
                    cc_in = dp.tile([128, 4], WDT, name="cc_in")
                    cc_out = dp.tile([NCORES, 128, 4], WDT, name="cc_out")
                    nc.scalar.dma_start(cc_in[:], hloc[:])
                    nc.gpsimd.collective_compute(
                        "AllGather",
                        mybir.AluOpType.bypass,
                        replica_groups=[list(range(NCORES))],
                        ins=[cc_in[:].opt()],
                        outs=[cc_out[:].opt()],
                    )
                    # cc_out[c, p, j] = h block col (c*4+j) partition p
                    nc.scalar.dma_start(
                        h_sb[:, 0:32].rearrange("p (c j) -> p c j", j=4),
                        cc_out[:].rearrange("c p j -> p c j"),
                    )
                    nc.vector.memset(h_sb[0:1, 32:33], 1.0)
                    nc.vector.tensor_copy(h_use[:], h_sb[:])
                # ------------------------------------------- head matmuls
                a2p = psB.tile([1, SH], F32, tag="a2p")
                a3p = psB.tile([1, SH], F32, tag="a3p")
                nh = len(head_chunks) // 2
                for ci, (t, kb0, nkb, width) in enumerate(head_chunks):
                    dst = a2p if ci < nh else a3p
                    for kk in range(nkb):
                        kb = kb0 + kk
                        nc.tensor.matmul(
                            dst[:],
                            h_use[:, kb : kb + 1],
                            head_tiles[ci][:, kk * SH : (kk + 1) * SH],
                            start=(kb == 0),
                            stop=(kb == KH - 1),
                        )

                a2row = ap.tile([1, SH], F32, tag="a2row")
                nc.scalar.activation(a2row[:], a2p[:], AF.Relu)
                a3row = ap.tile([1, SH], F32, tag="a3row")
                nc.scalar.activation(a3row[:], a3p[:], AF.Relu)

                # ---------------- a rows -> stationary cols (rank-1 PE)
                aT2 = psB.tile([128, 4], F32, tag="aT2")
                aT3 = psB.tile([128, 4], F32, tag="aT3")
                for k in range(4):
                    nc.tensor.matmul(
                        aT2[:, k : k + 1], a2row[0:1, ts(k, 128)], one,
                        start=True, stop=True,
                    )
                for k in range(4):
                    nc.tensor.matmul(
                        aT3[:, k : k + 1], a3row[0:1, ts(k, 128)], one,
                        start=True, stop=True,
                    )
                a_sb = ap.tile([128, 9], WDT, tag="astat")
                nc.vector.tensor_copy(a_sb[:, 0:4], aT2[:])
                nc.vector.tensor_copy(a_sb[:, 4:8], aT3[:])
                nc.vector.memset(a_sb[:, 8:9], 0.0)
                nc.vector.memset(a_sb[0:1, 8:9], 1.0)

                # ------------------------------------------- final gemvs
                op = psB.tile([1, 64], F32, tag="outp")
                cols2 = [0, 1, 2, 3, 8]
                cols3 = [4, 5, 6, 7, 8]
                for ki, k in enumerate(cols2):
                    nc.tensor.matmul(
                        op[:, 0:32],
                        a_sb[:, k : k + 1],
                        bw_sb[:, OFF_W2B + 32 * ki : OFF_W2B + 32 * (ki + 1)],
                        start=(ki == 0),
                        stop=(ki == KF - 1),
                    )
                for ki, k in enumerate(cols3):
                    nc.tensor.matmul(
                        op[:, 32:64],
                        a_sb[:, k : k + 1],
                        bw_sb[:, OFF_W3B + 32 * ki : OFF_W3B + 32 * (ki + 1)],
                        start=(ki == 0),
                        stop=(ki == KF - 1),
                    )
                out_sb = ap.tile([1, 64], F32, tag="osb")
                nc.scalar.activation(out_sb[:], op[:], AF.Copy)
                nc.gpsimd.dma_start(out[:], out_sb[:])
                if gather == "bcast2" and _RENDEZVOUS and _TAIL_READBACK:
                    # consume the alignment collective's output only here, at
                    # the tail of the gpsimd queue: keeps it from being pruned
                    # without any hot queue waiting on its completion. Target
                    # dead h_sb bytes so the h_sb dependency pins this after
                    # the gather.
                    nc.gpsimd.dma_start(h_sb[0:1, 0:8], bar_out[:])

    nc.compile()
    return nc


def _get_nc():
    key = (_WDT_NAME, _GDT_NAME, _HDT_NAME, _GATHER, _TAIL_READBACK, _RENDEZVOUS)
    if key not in _compiled:
        _compiled[key] = _build(_WDT_NAME, _GDT_NAME, _HDT_NAME, _GATHER)
    return _compiled[key]


# ------------------------------------------------------------------ host prep
def _pow2scale(*arrs):
    m = max(np.abs(np.asarray(a, np.float32)).max() for a in arrs)
    return float(2.0 ** np.floor(np.log2(FP8_TARGET / m)))


def _prep_in_maps(inputs):
    wnp = _np_wdt()
    f32 = np.float32
    gnp = {"bf16": ml_dtypes.bfloat16, "f8e3": ml_dtypes.float8_e3m4}[_GDT_NAME]
    hnp = {"bf16": ml_dtypes.bfloat16, "f8e3": ml_dtypes.float8_e3m4}[_HDT_NAME]

    def W(a):
        return np.ascontiguousarray(a, dtype=np.float32).astype(wnp)

    x = np.concatenate(
        [
            np.asarray(inputs[k], dtype=f32).ravel()
            for k in ("state_inno", "precov", "residual", "meas_cov")
        ]
    )
    W1 = np.asarray(inputs["W1"], f32)
    b1 = np.asarray(inputs["b1"], f32)
    w_ih = np.asarray(inputs["w_ih"], f32)
    w_hh = np.asarray(inputs["w_hh"], f32)
    b_ih = np.asarray(inputs["b_ih"], f32)
    b_hh = np.asarray(inputs["b_hh"], f32)
    h0 = np.asarray(inputs["h0"], f32)
    W2a = np.asarray(inputs["W2a"], f32)
    b2a = np.asarray(inputs["b2a"], f32)
    W2b = np.asarray(inputs["W2b"], f32)
    b2b = np.asarray(inputs["b2b"], f32)
    W3a = np.asarray(inputs["W3a"], f32)
    b3a = np.asarray(inputs["b3a"], f32)
    W3b = np.asarray(inputs["W3b"], f32)
    b3b = np.asarray(inputs["b3b"], f32)

    # fp8 (e3m4) quantization scales: weights are pre-scaled by s on the
    # host so the mantissa uses the format's range; the inverse folds into
    # the stationary activations (W1/b1 for gi, h0stat for gh, the hT4
    # rank-1 factor for the heads), so the kernel has zero descale ops.
    s_ih = _pow2scale(w_ih, b_ih) if _GDT_NAME == "f8e3" else 1.0
    s_hh = _pow2scale(w_hh, b_hh) if _GDT_NAME == "f8e3" else 1.0
    s_hd = _pow2scale(W2a, b2a, W3a, b3a) if _HDT_NAME == "f8e3" else 1.0

    # shared (core-independent) tensors, packed into two blobs:
    # blobw (weight dtype): w1t | h0stat | w2b | w3b | x    [128, CW]
    # blobf (f32):          b1s | one | hsc                 [128, CF]
    CW = H1P + KH + 2 * KF * 32 + 1
    CF = K1 + 2
    OFF_H0S = H1P
    OFF_W2B = H1P + KH
    OFF_W3B = H1P + KH + KF * 32
    OFF_X = CW - 1
    w1t = np.zeros((128, H1P), f32)
    w1t[:, :H1] = W1.T / s_ih
    h0stat = np.zeros((128, KH), f32)
    h0stat[:, :32] = h0.reshape(32, 128).T / s_hh
    h0stat[0, 32] = 1.0 / s_hh
    blobf = np.zeros((128, CF), f32)
    b1pad = np.zeros(H1P, f32)
    b1pad[:H1] = b1 / s_ih
    b1pad[H1] = 1.0 / s_ih  # aug element feeds the (scaled) bias rows of gi
    blobf[:, :K1] = b1pad.reshape(K1, 128).T
    blobf[0, K1] = 1.0  # "one"
    blobf[0, K1 + 1] = 1.0 / s_hd  # hsc: scales h for the fp8 head matmuls

    wihT = w_ih.T  # [H1, 3GH]
    whhT = w_hh.T  # [GH, 3GH]
    W2aT = W2a.T  # [GH, H2]
    W3aT = W3a.T

    in_maps = []
    for c in range(NCORES):
        s = slice(c * SH, (c + 1) * SH)
        gcols = np.r_[np.arange(c * SH, (c + 1) * SH),
                      np.arange(GH + c * SH, GH + (c + 1) * SH),
                      np.arange(2 * GH + c * SH, 2 * GH + (c + 1) * SH)]

        wiht = np.zeros((H1P, 3 * SH), f32)
        wiht[:H1, :] = wihT[:, gcols]
        wiht[H1, :] = b_ih[gcols]
        whht = np.zeros((GHP, 3 * SH), f32)
        whht[:GH, :] = whhT[:, gcols]
        whht[GH, :] = b_hh[gcols]
        w2at = np.zeros((GHP, SH), f32)
        w2at[:GH, :] = W2aT[:, s]
        w2at[GH, :] = b2a[s]
        w3at = np.zeros((GHP, SH), f32)
        w3at[:GH, :] = W3aT[:, s]
        w3at[GH, :] = b3a[s]

        def pack_stream(mat, nkb_total, step):
            width = mat.shape[1]
            blocks = []
            for kb0 in range(0, nkb_total, step):
                nkb = min(step, nkb_total - kb0)
                blk = (
                    mat[kb0 * 128 : (kb0 + nkb) * 128, :]
                    .reshape(nkb, 128, width)
                    .transpose(1, 0, 2)
                    .reshape(-1)
                )
                blocks.append(blk)
            return np.concatenate(blocks)

        wihp = pack_stream(wiht * s_ih, K1, GRU_CHUNK)
        whhp = pack_stream(whht * s_hh, KH, GRU_CHUNK)
        w2ap = pack_stream(w2at * s_hd, KH, HEAD_CHUNK)
        w3ap = pack_stream(w3at * s_hd, KH, HEAD_CHUNK)
        w2bt = np.zeros((KF * 128, 32), f32)
        w2bt[:SH, :] = W2b[:, s].T
        w3bt = np.zeros((KF * 128, 32), f32)
        w3bt[:SH, :] = W3b[:, s].T
        if c == 0:
            w2bt[SH, :] = b2b
            w3bt[SH, :] = b3b

        blobw = np.zeros((128, CW), f32)
        blobw[:, :H1P] = w1t
        blobw[:, OFF_H0S : OFF_H0S + KH] = h0stat
        # w2bt/w3bt rows (k*128+p, n) -> blob[p, OFF + k*32 + n]
        blobw[:, OFF_W2B : OFF_W2B + KF * 32] = (
            w2bt.reshape(KF, 128, 32).transpose(1, 0, 2).reshape(128, KF * 32)
        )
        blobw[:, OFF_W3B : OFF_W3B + KF * 32] = (
            w3bt.reshape(KF, 128, 32).transpose(1, 0, 2).reshape(128, KF * 32)
        )
        blobw[:, OFF_X] = x

        in_maps.append(
            {
                "blobw": blobw.astype(wnp),
                "blobf": blobf,
                "wihp": wihp.astype(gnp),
                "whhp": whhp.astype(gnp),
                "h0row": h0[s].reshape(1, SH),
                "w2ap": w2ap.astype(hnp),
                "w3ap": w3ap.astype(hnp),
                "coreid": np.array([[c]], dtype=np.uint32),
            }
        )
    return in_maps


def run(inputs, trace=False):
    from concourse.bass_utils import run_bass_kernel_spmd

    nc = _get_nc()
    in_maps = _prep_in_maps(inputs)
    res = run_bass_kernel_spmd(
        nc, in_maps, core_ids=list(range(NCORES)), trace=trace
    )
    total = np.sum([np.asarray(r["out"], np.float64) for r in res.results], axis=0)
    total = total.astype(np.float32).ravel()
    x_hat = total[:32].reshape(X_DIM, 1)
    P_hat = total[32:].reshape(X_DIM, 1)
    return (x_hat, P_hat), res


def kernel(**inputs):
    (x_hat, P_hat), _ = run(inputs, trace=False)
    return (x_hat, P_hat)



# revision 24
# speedup vs baseline: 50.5378x; 50.5378x over previous
"""Trainium2 8-core Bass kernel for the SKalmanNet dense-MLP GEMV chain.

Network (batch=1):
  x   = concat(state_inno, precov, residual, meas_cov)          [128]
  l1  = relu(W1 @ x + b1)                                       [1344]
  gi  = w_ih @ l1 + b_ih ; gh = w_hh @ h0 + b_hh                [12288]
  r,z = sigmoid(gi+gh) gates ; n = tanh(gi_n + r*gh_n)
  h   = (1-z)*n + z*h0                                          [4096]
  x_hat = W2b @ relu(W2a @ h + b2a) + b2b                       [32]
  P_hat = W3b @ relu(W3a @ h + b3a) + b3b                       [32]

Sharding: every large matrix is row-sharded (output dim) across 8 cores;
W1 is replicated (tiny) so l1 needs no collective. The only collective is
one 16KB AllGather of h. The final 32-vector partials (W2b/W3b column
shards) are summed on the host during unsharding.

Layouts: activations live as "stationary" columns [128, nblk] so they can
be the matmul lhsT; weights are host-pre-transposed so W.T tiles stream
as the rhs. All biases are folded into the matmuls via an augmented
contraction element that is constant 1.
"""

import os
import sys

sys.path.insert(0, "/opt/trn_rl_repo")

import numpy as np
import ml_dtypes

# ---------------------------------------------------------------- constants
NCORES = 8
X_DIM = 32
IN2 = 128                      # l1 input dim
H1 = 1344                      # l1 output / GRU input dim
H1P = 1408                     # padded to 11*128 (pad block holds the bias row)
GH = 4096                      # GRU hidden
GHP = 4224                     # padded to 33*128 (aug block holds bias row)
H2 = 4096                      # head hidden
SH = 512                       # per-core hidden slice (GH/8 == H2/8)
K1 = H1P // 128                # 11 contraction blocks for gi
KH = GHP // 128                # 33 contraction blocks for gh / heads
KF = 640 // 128                # 5 contraction blocks for the final gemv

GRU_CHUNK = 4                  # k-blocks per DMA chunk for wiht/whht
HEAD_CHUNK = 8                 # k-blocks per DMA chunk for w2at/w3at

_WDT_NAME = os.environ.get("KERNEL_DTYPE", "bf16")
_GDT_NAME = os.environ.get("KERNEL_GRU_DT", "f8e3")     # bf16 | f8e3
_HDT_NAME = os.environ.get("KERNEL_HEAD_DT", "f8e3")    # bf16 | f8e3
_GATHER = os.environ.get("KERNEL_GATHER", "bcast2")
_EARLY_PREP = os.environ.get("KERNEL_EARLY_PREP", "0") == "1"
_TAIL_READBACK = os.environ.get("KERNEL_TAIL_READBACK", "1") == "1"
_RENDEZVOUS = os.environ.get("KERNEL_RENDEZVOUS", "1") == "1"
FP8_TARGET = 12.0               # quantization scale target (e3m4 max 15.5)

_compiled = {}


def _np_wdt():
    return {"bf16": ml_dtypes.bfloat16, "f32": np.float32, "f32r": np.float32}[
        _WDT_NAME
    ]


def _build(wdt_name, gdt_name, hdt_name, gather):
    import concourse.bass as bass  # noqa: F401
    import concourse.mybir as mybir
    import concourse.tile as tile
    from concourse import bacc

    F32 = mybir.dt.float32
    WDT = {
        "bf16": mybir.dt.bfloat16,
        "f32": mybir.dt.float32,
        "f32r": mybir.dt.float32r,
    }[wdt_name]
    GDT = {"bf16": mybir.dt.bfloat16, "f8e3": mybir.dt.float8e3}[gdt_name]
    HDT = {"bf16": mybir.dt.bfloat16, "f8e3": mybir.dt.float8e3}[hdt_name]
    AF = mybir.ActivationFunctionType
    ALU = mybir.AluOpType
    ts = bass.ts

    nc = bacc.Bacc("TRN2", target_bir_lowering=False, debug=False, num_devices=NCORES)

    # ------------------------------------------------------------- I/O decl
    # All small constants ride in two packed blobs (per-partition contiguous)
    # so they cost ~256 DMA descriptors instead of ~2000 tiny ones that
    # starve behind the weight stream on the shared DMA engines.
    CW = H1P + KH + 2 * KF * 32 + 1      # w1t | h0stat | w2b | w3b | x
    CF = K1 + 2                          # b1s | one | hsc
    OFF_H0S = H1P
    OFF_W2B = H1P + KH
    OFF_W3B = H1P + KH + KF * 32
    OFF_X = CW - 1
    blobw = nc.dram_tensor("blobw", [128, CW], WDT, kind="ExternalInput")
    blobf = nc.dram_tensor("blobf", [128, CF], F32, kind="ExternalInput")
    # packed streams: per chunk, [128, nkb*N] with per-partition-contiguous
    # bytes so each DMA descriptor is one long run
    wihp = nc.dram_tensor("wihp", [K1 * 128 * 3 * SH], GDT, kind="ExternalInput")
    whhp = nc.dram_tensor("whhp", [KH * 128 * 3 * SH], GDT, kind="ExternalInput")
    w2ap = nc.dram_tensor("w2ap", [KH * 128 * SH], HDT, kind="ExternalInput")
    w3ap = nc.dram_tensor("w3ap", [KH * 128 * SH], HDT, kind="ExternalInput")
    h0row = nc.dram_tensor("h0row", [1, SH], F32, kind="ExternalInput")
    coreid = nc.dram_tensor("coreid", [1, 1], mybir.dt.uint32, kind="ExternalInput")
    out = nc.dram_tensor("out", [1, 64], F32, kind="ExternalOutput")

    def chunks_of(t, nkb_total, step, width):
        out = []
        for kb0 in range(0, nkb_total, step):
            out.append((t, kb0, min(step, nkb_total - kb0), width))
        return out

    gru_chunks = chunks_of(wihp, K1, GRU_CHUNK, 3 * SH) + chunks_of(
        whhp, KH, GRU_CHUNK, 3 * SH
    )
    n_wih_chunks = len(chunks_of(wihp, K1, GRU_CHUNK, 3 * SH))
    head_chunks = chunks_of(w2ap, KH, HEAD_CHUNK, SH) + chunks_of(
        w3ap, KH, HEAD_CHUNK, SH
    )

    with tile.TileContext(nc) as tc:
        with (
            tc.tile_pool(name="const", bufs=1) as cp,
            tc.tile_pool(name="gru", bufs=12) as gp,
            tc.tile_pool(name="head", bufs=10) as hp,
            tc.tile_pool(name="acts", bufs=1) as ap,
            tc.tile_pool(name="dram", bufs=1, space="DRAM") as dp,
        ):
            # ------------------------------------------------ constant loads
            # Constants go on the ACT HWDGE ring so the SP ring starts the
            # big weight stream immediately.
            bw_sb = cp.tile([128, CW], WDT, tag="bw")
            nc.scalar.dma_start(bw_sb[:], blobw[:])
            bf_sb = cp.tile([128, CF], F32, tag="bf")
            nc.scalar.dma_start(bf_sb[:], blobf[:])
            h0r_sb = cp.tile([1, SH], F32, tag="h0r")
            nc.scalar.dma_start(h0r_sb[:], h0row[:])
            cid_sb = cp.tile([1, 1], mybir.dt.uint32, tag="cid")
            nc.scalar.dma_start(cid_sb[:], coreid[:])
            def ncfw_rendezvous(read_back):
                # All-core ncfw AllReduce. Its presence in the NEFF makes the
                # runtime align the 8 cores' launches (without it they stagger
                # by ~100us/core). ncfw's CC core has a ~65us wake latency
                # after the trigger: trigger at program start, and (bcast2)
                # read nothing back so no queue ever blocks on completion —
                # the mesh completes in the background well before the NEFF
                # drains.
                bar_sb = cp.tile([1, 8], mybir.dt.uint32, tag="bar")
                nc.vector.memset(bar_sb[:], 1)
                bar_in = dp.tile([1, 8], mybir.dt.uint32, name="bar_in")
                bar_out = dp.tile([1, 8], mybir.dt.uint32, name="bar_out")
                if _RENDEZVOUS:
                    # bar_in rides the ACT HWDGE ring: a gpsimd (SWDGE) dma
                    # here posts its completion sem only at ~60-74us, and the
                    # collective enqueue + the gather critical queue up behind
                    # it on the Pool queue.
                    nc.scalar.dma_start(bar_in[:], bar_sb[:])
                    nc.gpsimd.collective_compute(
                        "AllReduce",
                        mybir.AluOpType.add,
                        replica_groups=[list(range(NCORES))],
                        ins=[bar_in[:].opt()],
                        outs=[bar_out[:].opt()],
                    )
                bar2_sb = cp.tile([1, 8], mybir.dt.uint32, tag="bar2")
                if read_back:
                    # cc/bcast: gather logic reads bar2 -> load now (gpsimd).
                    nc.gpsimd.dma_start(bar2_sb[:], bar_out[:])
                return bar_out, bar2_sb

            # gather target: written remotely by all 8 cores' broadcasts.
            # memset early so the slot is reserved for the whole kernel and
            # cannot alias a streaming tile when a peer's write lands.
            h_sb = ap.tile([128, KH], WDT, tag="hstat")
            hloc = ap.tile([128, 4], WDT, tag="hloc")
            if gather == "bcast2":
                # ncfw-free gather. Startup: zero h_sb, then check in with all
                # peers via a remote-sem broadcast (the flag barrier), and
                # pre-generate the h-broadcast descriptors (If-ladder) OFF the
                # critical path. bass defers the prep's data deps to the
                # trigger, so at gather time only trigger+transfer remain.
                # The alignment collective is emitted first so its trigger
                # lands at ~9us; nothing reads its output until the gpsimd
                # queue tail.
                # NOTE: ncfw_rendezvous is emitted AFTER the gather critical
                # (below) — the InstCollectiveCompute only retires when the
                # CC core picks it up (~52-65us wake), and anything behind it
                # on the Pool queue (the gather critical!) stalls that long.
                bar_r = nc.alloc_semaphore("bar_r")
                bar_l = nc.alloc_semaphore("bar_l")
                bar_p = nc.alloc_semaphore("bar_prep_sem")
                psem = nc.alloc_semaphore("bc_prep_sem")
                lsem = nc.alloc_semaphore("bc_local_sem")
                rsem = nc.alloc_semaphore("bc_remote_sem")
                nc.vector.memset(hloc[:], 0.0)
                nc.vector.memset(h_sb[:], 0.0)
                # NOTE: no startup tile_critical — a critical here makes every
                # later instruction (via pool boundaries) wait on the Pool
                # queue's exit, and the gpsimd SWDGE ucode library load inside
                # costs ~18us, stalling the PE start at ~32us. The flag
                # barrier moved into the gather critical below.
                if _EARLY_PREP:
                    with tc.tile_critical(sync_engine=mybir.EngineType.Pool, no_gpsimd_drain=True):
                        eng = nc.gpsimd
                        reg = eng.alloc_register("cid_reg")
                        eng.reg_load(reg, cid_sb[0:1, 0:1])
                        for c in range(NCORES):
                            with eng.If_eq(reg, c):
                                eng.remote_dma_broadcast(
                                    out_ap=h_sb[:, c * 4 : (c + 1) * 4],
                                    in_ap=hloc[:],
                                    remote_sem=rsem,
                                    local_sem=lsem,
                                    rdests=[(0, k) for k in range(NCORES)],
                                ).then_inc(psem, 1)
                            with eng.Else():
                                eng.nop()
            else:
                nc.vector.memset(h_sb[:], 0.0)


            if gather != "bcast2":
                bar_out, bar2_sb = ncfw_rendezvous(read_back=True)

            # ------------------------------------- weight stream DMAs (HWDGE)
            # Consumption order: wih -> whh -> heads. The GRU (and hence the
            # gather) completes as early as possible; the gather + gate tail
            # hides under the head-weight stream.
            def stream_chunk(pool, spec, tag, engine, dt):
                t, kb0, nkb, width = spec
                g = pool.tile([128, GRU_CHUNK * 3 * SH] if width == 3 * SH
                              else [128, HEAD_CHUNK * SH], dt, tag=tag, name=tag)
                off = kb0 * 128 * width
                sz = nkb * 128 * width
                src_ap = t[off : off + sz].rearrange("(p x) -> p x", p=128)
                engine.dma_start(g[:, 0 : nkb * width], src_ap)
                return g

            gru_tiles = []
            for spec in gru_chunks:
                gru_tiles.append(stream_chunk(gp, spec, "gruw", nc.sync, GDT))
            head_tiles = []
            for spec in head_chunks:
                head_tiles.append(stream_chunk(hp, spec, "headw", nc.sync, HDT))

            with tc.tile_pool(name="psA", bufs=1, space="PSUM") as psA:
                # ------------------------------------------- L1 (W-stationary)
                l1p = psA.tile([128, K1], F32, tag="l1p")
                for j in range(K1):
                    nc.tensor.matmul(
                        l1p[:, j : j + 1],
                        bw_sb[:, ts(j, 128)],
                        bw_sb[:, OFF_X : OFF_X + 1],
                        start=True,
                        stop=True,
                    )
                l1t = ap.tile([128, K1], F32, tag="l1t")
                nc.vector.scalar_tensor_tensor(
                    l1t[:], l1p[:], 1.0, bf_sb[:, 0:K1], ALU.mult, ALU.add
                )
                l1_sb = ap.tile([128, K1], WDT, tag="l1s")
                nc.scalar.activation(l1_sb[:], l1t[:], AF.Relu)

                # ------------------------------------------- GRU matmuls
                gi = [psA.tile([1, SH], F32, tag=f"gi{g}", name=f"gi{g}") for g in range(3)]
                gh = [psA.tile([1, SH], F32, tag=f"gh{g}", name=f"gh{g}") for g in range(3)]
                for ci, (t, kb0, nkb, width) in enumerate(gru_chunks):
                    is_ih = ci < n_wih_chunks
                    dst = gi if is_ih else gh
                    klast = (K1 if is_ih else KH) - 1
                    for kk in range(nkb):
                        kb = kb0 + kk
                        stat = (
                            l1_sb[:, kb : kb + 1]
                            if is_ih
                            else bw_sb[:, OFF_H0S + kb : OFF_H0S + kb + 1]
                        )
                        for g in range(3):
                            base = kk * width + g * SH
                            nc.tensor.matmul(
                                dst[g][:],
                                stat,
                                gru_tiles[ci][:, base : base + SH],
                                start=(kb == 0),
                                stop=(kb == klast),
                            )

                # gi -> SBUF (ScalarE; overlaps the gh matmul stream). DVE has
                # a single PSUM read port, so gate ops may touch <=1 PSUM operand.
                gis = ap.tile([1, 3 * SH], F32, tag="gis")
                for g in range(3):
                    nc.scalar.activation(gis[:, ts(g, SH)], gi[g][:], AF.Copy)

                # ------------------------------------------- gates (row layout)
                t_r = ap.tile([1, SH], F32, tag="gtmp", bufs=6)
                nc.vector.tensor_tensor(t_r[:], gis[:, ts(0, SH)], gh[0][:], ALU.add)
                r = ap.tile([1, SH], F32, tag="r")
                nc.scalar.activation(r[:], t_r[:], AF.Sigmoid)
                t_z = ap.tile([1, SH], F32, tag="gtmp", bufs=6)
                nc.vector.tensor_tensor(t_z[:], gis[:, ts(1, SH)], gh[1][:], ALU.add)
                z = ap.tile([1, SH], F32, tag="z")
                nc.scalar.activation(z[:], t_z[:], AF.Sigmoid)
                t_m = ap.tile([1, SH], F32, tag="gtmp", bufs=6)
                nc.vector.tensor_tensor(t_m[:], r[:], gh[2][:], ALU.mult)
                t_n = ap.tile([1, SH], F32, tag="gtmp", bufs=6)
                nc.vector.tensor_tensor(t_n[:], t_m[:], gis[:, ts(2, SH)], ALU.add)
                n_t = ap.tile([1, SH], F32, tag="n")
                nc.scalar.activation(n_t[:], t_n[:], AF.Tanh)
                t_d = ap.tile([1, SH], F32, tag="gtmp", bufs=6)
                nc.vector.tensor_tensor(t_d[:], h0r_sb[:], n_t[:], ALU.subtract)
                t_e = ap.tile([1, SH], F32, tag="gtmp", bufs=6)
                nc.vector.tensor_tensor(t_e[:], z[:], t_d[:], ALU.mult)
                h_row = ap.tile([1, SH], F32, tag="hrow")
                nc.vector.tensor_tensor(h_row[:], n_t[:], t_e[:], ALU.add)

            # ------------- h row -> stationary cols via rank-1 PE matmuls
            one = bf_sb[0:1, K1 : K1 + 1]  # constant 1.0
            hsc = bf_sb[0:1, K1 + 1 : K1 + 2]  # 1/s_head (1.0 unless heads fp8)
            with tc.tile_pool(name="psB", bufs=1, space="PSUM") as psB:
                hT4 = psB.tile([128, 4], F32, tag="hT4")
                for k in range(4):
                    nc.tensor.matmul(
                        hT4[:, k : k + 1],
                        h_row[0:1, ts(k, 128)],
                        hsc,
                        start=True,
                        stop=True,
                    )
                nc.vector.tensor_copy(hloc[:], hT4[:])

                # ---------------- all-gather h across the 8 cores
                h_use = ap.tile([128, KH], WDT, tag="huse")
                if gather == "bcast2" and _EARLY_PREP:
                    with tc.tile_critical(sync_engine=mybir.EngineType.Pool, no_gpsimd_drain=True):
                        eng = nc.gpsimd
                        eng.wait_ge(psem, 1)
                        eng.wait_ge(bar_r, 16)
                        eng.trigger_dma(count=1)
                        eng.wait_ge(lsem, 16)
                        eng.wait_ge(rsem, 16)
                        eng.tensor_copy(h_sb[0:1, 32:33], hsc)  # aug = 1/s_head
                    nc.vector.tensor_copy(h_use[:], h_sb[:])
                elif gather == "bcast2":
                    with tc.tile_critical(sync_engine=mybir.EngineType.Pool, no_gpsimd_drain=True):
                        eng = nc.gpsimd
                        # flag barrier: check in with all peers, then wait for
                        # everyone before firing the h broadcast (h_sb on all
                        # peers is long since zeroed by the early memset).
                        eng.remote_sem_update_broadcast(
                            bar_r, bar_l, rdests=[(0, k) for k in range(NCORES)]
                        ).then_inc(bar_p, 1)
                        eng.wait_ge(bar_p, 1)
                        eng.trigger_dma(count=1)
                        reg = eng.alloc_register("cid_reg")
                        eng.reg_load(reg, cid_sb[0:1, 0:1])
                        for c in range(NCORES):
                            with eng.If_eq(reg, c):
                                eng.remote_dma_broadcast(
                                    out_ap=h_sb[:, c * 4 : (c + 1) * 4],
                                    in_ap=hloc[:],
                                    remote_sem=rsem,
                                    local_sem=lsem,
                                    rdests=[(0, k) for k in range(NCORES)],
                                ).then_inc(psem, 1)
                            with eng.Else():
                                eng.nop()
                        eng.wait_ge(psem, 1)
                        eng.wait_ge(bar_r, 16)
                        eng.trigger_dma(count=1)
                        eng.wait_ge(lsem, 16)
                        eng.wait_ge(rsem, 16)
                        eng.tensor_copy(h_sb[0:1, 32:33], hsc)  # aug = 1/s_head
                    nc.vector.tensor_copy(h_use[:], h_sb[:])
                    bar_out, bar2_sb = ncfw_rendezvous(read_back=False)
                elif gather == "bcast":
                    psem = nc.alloc_semaphore("bc_prep_sem")
                    lsem = nc.alloc_semaphore("bc_local_sem")
                    rsem = nc.alloc_semaphore("bc_remote_sem")
                    with tc.tile_critical():
                        eng = nc.gpsimd
                        reg = eng.alloc_register("cid_reg")
                        # order after the startup alignment barrier
                        eng.reg_load(reg, bar2_sb[0:1, 0:1])
                        eng.reg_load(reg, cid_sb[0:1, 0:1])
                        for c in range(NCORES):
                            with eng.If_eq(reg, c):
                                eng.remote_dma_broadcast(
                                    out_ap=h_sb[:, c * 4 : (c + 1) * 4],
                                    in_ap=hloc[:],
                                    remote_sem=rsem,
                                    local_sem=lsem,
                                    rdests=[(0, k) for k in range(NCORES)],
                                ).then_inc(psem, 1)
                            with eng.Else():
                                eng.nop()
                        eng.wait_ge(psem, 1)
                        eng.trigger_dma(count=1)
                        eng.wait_ge(lsem, 16)
                        eng.wait_ge(rsem, 16)
                        eng.memset(h_sb[0:1, 32:33], 1.0)  # aug element
                        # copy into h_use so downstream consumers depend on
                        # the gathered data (remote writes invisible to Tile)
                        eng.tensor_copy(h_use[:], h_sb[:])
                else:
                    cc_in = dp.tile([128, 4], WDT, name="cc_in")
                    cc_out = dp.tile([NCORES, 128, 4], WDT, name="cc_out")
                    nc.scalar.dma_start(cc_in[:], hloc[:])
                    nc.gpsimd.collective_compute(
                        "AllGather",
                        mybir.AluOpType.bypass,
                        replica_groups=[list(range(NCORES))],
                        ins=[cc_in[:].opt()],
                        outs=[cc_out[:].opt()],
                    )
                    # cc_out[c, p, j] = h block col (c*4+j) partition p
                    nc.scalar.dma_start(
                        h_sb[:, 0:32].rearrange("p (c j) -> p c j", j=4),
                        cc_out[:].rearrange("c p j -> p c j"),
                    )
                    nc.vector.memset(h_sb[0:1, 32:33], 1.0)
                    nc.vector.tensor_copy(h_use[:], h_sb[:])
                # ------------------------------------------- head matmuls
                a2p = psB.tile([1, SH], F32, tag="a2p")
                a3p = psB.tile([1, SH], F32, tag="a3p")
                nh = len(head_chunks) // 2
                for ci, (t, kb0, nkb, width) in enumerate(head_chunks):
                    dst = a2p if ci < nh else a3p
                    for kk in range(nkb):
                        kb = kb0 + kk
                        nc.tensor.matmul(
                            dst[:],
                            h_use[:, kb : kb + 1],
                            head_tiles[ci][:, kk * SH : (kk + 1) * SH],
                            start=(kb == 0),
                            stop=(kb == KH - 1),
                        )

                a2row = ap.tile([1, SH], F32, tag="a2row")
                nc.scalar.activation(a2row[:], a2p[:], AF.Relu)
                a3row = ap.tile([1, SH], F32, tag="a3row")
                nc.scalar.activation(a3row[:], a3p[:], AF.Relu)

                # ---------------- a rows -> stationary cols (rank-1 PE)
                aT2 = psB.tile([128, 4], F32, tag="aT2")
                aT3 = psB.tile([128, 4], F32, tag="aT3")
                for k in range(4):
                    nc.tensor.matmul(
                        aT2[:, k : k + 1], a2row[0:1, ts(k, 128)], one,
                        start=True, stop=True,
                    )
                for k in range(4):
                    nc.tensor.matmul(
                        aT3[:, k : k + 1], a3row[0:1, ts(k, 128)], one,
                        start=True, stop=True,
                    )
                a_sb = ap.tile([128, 9], WDT, tag="astat")
                nc.vector.tensor_copy(a_sb[:, 0:4], aT2[:])
                nc.vector.tensor_copy(a_sb[:, 4:8], aT3[:])
                nc.vector.memset(a_sb[:, 8:9], 0.0)
                nc.vector.memset(a_sb[0:1, 8:9], 1.0)

                # ------------------------------------------- final gemvs
                op = psB.tile([1, 64], F32, tag="outp")
                cols2 = [0, 1, 2, 3, 8]
                cols3 = [4, 5, 6, 7, 8]
                for ki, k in enumerate(cols2):
                    nc.tensor.matmul(
                        op[:, 0:32],
                        a_sb[:, k : k + 1],
                        bw_sb[:, OFF_W2B + 32 * ki : OFF_W2B + 32 * (ki + 1)],
                        start=(ki == 0),
                        stop=(ki == KF - 1),
                    )
                for ki, k in enumerate(cols3):
                    nc.tensor.matmul(
                        op[:, 32:64],
                        a_sb[:, k : k + 1],
                        bw_sb[:, OFF_W3B + 32 * ki : OFF_W3B + 32 * (ki + 1)],
                        start=(ki == 0),
                        stop=(ki == KF - 1),
                    )
                out_sb = ap.tile([1, 64], F32, tag="osb")
                nc.scalar.activation(out_sb[:], op[:], AF.Copy)
                nc.gpsimd.dma_start(out[:], out_sb[:])
                if gather == "bcast2" and _RENDEZVOUS and _TAIL_READBACK:
                    # consume the alignment collective's output only here, at
                    # the tail of the gpsimd queue: keeps it from being pruned
                    # without any hot queue waiting on its completion. Target
                    # dead h_sb bytes so the h_sb dependency pins this after
                    # the gather.
                    nc.gpsimd.dma_start(h_sb[0:1, 0:8], bar_out[:])

    nc.compile()
    return nc


def _get_nc():
    key = (_WDT_NAME, _GDT_NAME, _HDT_NAME, _GATHER, _TAIL_READBACK, _RENDEZVOUS)
    if key not in _compiled:
        _compiled[key] = _build(_WDT_NAME, _GDT_NAME, _HDT_NAME, _GATHER)
    return _compiled[key]


# ------------------------------------------------------------------ host prep
def _pow2scale(*arrs):
    m = max(np.abs(np.asarray(a, np.float32)).max() for a in arrs)
    return float(2.0 ** np.floor(np.log2(FP8_TARGET / m)))


def _prep_in_maps(inputs):
    wnp = _np_wdt()
    f32 = np.float32
    gnp = {"bf16": ml_dtypes.bfloat16, "f8e3": ml_dtypes.float8_e3m4}[_GDT_NAME]
    hnp = {"bf16": ml_dtypes.bfloat16, "f8e3": ml_dtypes.float8_e3m4}[_HDT_NAME]

    def W(a):
        return np.ascontiguousarray(a, dtype=np.float32).astype(wnp)

    x = np.concatenate(
        [
            np.asarray(inputs[k], dtype=f32).ravel()
            for k in ("state_inno", "precov", "residual", "meas_cov")
        ]
    )
    W1 = np.asarray(inputs["W1"], f32)
    b1 = np.asarray(inputs["b1"], f32)
    w_ih = np.asarray(inputs["w_ih"], f32)
    w_hh = np.asarray(inputs["w_hh"], f32)
    b_ih = np.asarray(inputs["b_ih"], f32)
    b_hh = np.asarray(inputs["b_hh"], f32)
    h0 = np.asarray(inputs["h0"], f32)
    W2a = np.asarray(inputs["W2a"], f32)
    b2a = np.asarray(inputs["b2a"], f32)
    W2b = np.asarray(inputs["W2b"], f32)
    b2b = np.asarray(inputs["b2b"], f32)
    W3a = np.asarray(inputs["W3a"], f32)
    b3a = np.asarray(inputs["b3a"], f32)
    W3b = np.asarray(inputs["W3b"], f32)
    b3b = np.asarray(inputs["b3b"], f32)

    # fp8 (e3m4) quantization scales: weights are pre-scaled by s on the
    # host so the mantissa uses the format's range; the inverse folds into
    # the stationary activations (W1/b1 for gi, h0stat for gh, the hT4
    # rank-1 factor for the heads), so the kernel has zero descale ops.
    s_ih = _pow2scale(w_ih, b_ih) if _GDT_NAME == "f8e3" else 1.0
    s_hh = _pow2scale(w_hh, b_hh) if _GDT_NAME == "f8e3" else 1.0
    s_hd = _pow2scale(W2a, b2a, W3a, b3a) if _HDT_NAME == "f8e3" else 1.0

    # shared (core-independent) tensors, packed into two blobs:
    # blobw (weight dtype): w1t | h0stat | w2b | w3b | x    [128, CW]
    # blobf (f32):          b1s | one | hsc                 [128, CF]
    CW = H1P + KH + 2 * KF * 32 + 1
    CF = K1 + 2
    OFF_H0S = H1P
    OFF_W2B = H1P + KH
    OFF_W3B = H1P + KH + KF * 32
    OFF_X = CW - 1
    w1t = np.zeros((128, H1P), f32)
    w1t[:, :H1] = W1.T / s_ih
    h0stat = np.zeros((128, KH), f32)
    h0stat[:, :32] = h0.reshape(32, 128).T / s_hh
    h0stat[0, 32] = 1.0 / s_hh
    blobf = np.zeros((128, CF), f32)
    b1pad = np.zeros(H1P, f32)
    b1pad[:H1] = b1 / s_ih
    b1pad[H1] = 1.0 / s_ih  # aug element feeds the (scaled) bias rows of gi
    blobf[:, :K1] = b1pad.reshape(K1, 128).T
    blobf[0, K1] = 1.0  # "one"
    blobf[0, K1 + 1] = 1.0 / s_hd  # hsc: scales h for the fp8 head matmuls

    wihT = w_ih.T  # [H1, 3GH]
    whhT = w_hh.T  # [GH, 3GH]
    W2aT = W2a.T  # [GH, H2]
    W3aT = W3a.T

    in_maps = []
    for c in range(NCORES):
        s = slice(c * SH, (c + 1) * SH)
        gcols = np.r_[np.arange(c * SH, (c + 1) * SH),
                      np.arange(GH + c * SH, GH + (c + 1) * SH),
                      np.arange(2 * GH + c * SH, 2 * GH + (c + 1) * SH)]

        wiht = np.zeros((H1P, 3 * SH), f32)
        wiht[:H1, :] = wihT[:, gcols]
        wiht[H1, :] = b_ih[gcols]
        whht = np.zeros((GHP, 3 * SH), f32)
        whht[:GH, :] = whhT[:, gcols]
        whht[GH, :] = b_hh[gcols]
        w2at = np.zeros((GHP, SH), f32)
        w2at[:GH, :] = W2aT[:, s]
        w2at[GH, :] = b2a[s]
        w3at = np.zeros((GHP, SH), f32)
        w3at[:GH, :] = W3aT[:, s]
        w3at[GH, :] = b3a[s]

        def pack_stream(mat, nkb_total, step):
            width = mat.shape[1]
            blocks = []
            for kb0 in range(0, nkb_total, step):
                nkb = min(step, nkb_total - kb0)
                blk = (
                    mat[kb0 * 128 : (kb0 + nkb) * 128, :]
                    .reshape(nkb, 128, width)
                    .transpose(1, 0, 2)
                    .reshape(-1)
                )
                blocks.append(blk)
            return np.concatenate(blocks)

        wihp = pack_stream(wiht * s_ih, K1, GRU_CHUNK)
        whhp = pack_stream(whht * s_hh, KH, GRU_CHUNK)
        w2ap = pack_stream(w2at * s_hd, KH, HEAD_CHUNK)
        w3ap = pack_stream(w3at * s_hd, KH, HEAD_CHUNK)
        w2bt = np.zeros((KF * 128, 32), f32)
        w2bt[:SH, :] = W2b[:, s].T
        w3bt = np.zeros((KF * 128, 32), f32)
        w3bt[:SH, :] = W3b[:, s].T
        if c == 0:
            w2bt[SH, :] = b2b
            w3bt[SH, :] = b3b

        blobw = np.zeros((128, CW), f32)
        blobw[:, :H1P] = w1t
        blobw[:, OFF_H0S : OFF_H0S + KH] = h0stat
        # w2bt/w3bt rows (k*128+p, n) -> blob[p, OFF + k*32 + n]
        blobw[:, OFF_W2B : OFF_W2B + KF * 32] = (
            w2bt.reshape(KF, 128, 32).transpose(1, 0, 2).reshape(128, KF * 32)
        )
        blobw[:, OFF_W3B : OFF_W3B + KF * 32] = (
            w3bt.reshape(KF, 128, 32).transpose(1, 0, 2).reshape(128, KF * 32)
        )
        blobw[:, OFF_X] = x

        in_maps.append(
            {
                "blobw": blobw.astype(wnp),
                "blobf": blobf,
                "wihp": wihp.astype(gnp),
                "whhp": whhp.astype(gnp),
                "h0row": h0[s].reshape(1, SH),
                "w2ap": w2ap.astype(hnp),
                "w3ap": w3ap.astype(hnp),
                "coreid": np.array([[c]], dtype=np.uint32),
            }
        )
    return in_maps


def run(inputs, trace=False):
    from concourse.bass_utils import run_bass_kernel_spmd

    nc = _get_nc()
    in_maps = _prep_in_maps(inputs)
    res = run_bass_kernel_spmd(
        nc, in_maps, core_ids=list(range(NCORES)), trace=trace
    )
    total = np.sum([np.asarray(r["out"], np.float64) for r in res.results], axis=0)
    total = total.astype(np.float32).ravel()
    x_hat = total[:32].reshape(X_DIM, 1)
    P_hat = total[32:].reshape(X_DIM, 1)
    return (x_hat, P_hat), res


def kernel(**inputs):
    (x_hat, P_hat), _ = run(inputs, trace=False)
    return (x_hat, P_hat)

